# revision 46
# baseline (speedup 1.0000x reference)
"""CoAttention ImageDNS kernel for Trainium2 (8 NeuronCores, Bass/Tile).

Math: the reference computes two additive-attention blocks. In both, the
softmax'd score is  score[b, q, k] = f(q-side)[b, q] + g(k-side)[b, k] + c,
and softmax over k is invariant to the q-dependent (and constant) terms, so
the attention weights are independent of the query index:

  visual_att[b, s, :]  = softmax_r( wB . tanh(W_i1 @ img[b, r]) )
  textual_att[b, i, :] = softmax_j( wD . tanh(W_d2 @ dns[b, j]) )

Hence both outputs are per-batch rank-1 broadcasts:

  att_img_features[b, s, :] = visual_att[b]  @ img[b]   (same for all s)
  att_dns_features[b, i, :] = textual_att[b] @ dns[b]   (same for all i)

W_d1/b_d1/w_att1[:H]/b_att1/W_i2/b_i2/w_att2[:H]/b_att2 cancel entirely.

Sharding: pure data-parallel over batch, 4 batches per core, no collectives.
The device computes the per-batch [H] attention outputs; the host broadcasts
them over the (identical) S query rows, so the kernel writes only B*H values
instead of B*S*H.

Layout: projections keep activations stationary (lhsT = x^T chunk, reused
across both 512-wide output halves, so LDWEIGHTS stays hidden) and stream
the weights; proj rows land on PSUM partitions.  Scores: tanh (Scalar, bf16)
then a weighted free-dim reduction (Vector stt accum); exp'd score columns
are broadcast to [128, rows] with one tiny PE matmul per chunk
(a-column x identity), softmax sums come free from a Scalar Copy+accum over
the broadcast tile, and stage-2 weighted row sums run on Vector over the
same transposed activation tiles the projections use (the natural-layout
activations are never loaded).  Each group's tail is emitted one group later
and split into parts spread between the next group's score reductions; rows
are packed across batches (img: all 4 batches = 7 row chunks) to minimize
M-padding.  The final dns batch instead runs its softmax sum and stage-2 as
PE matmuls over natural-layout rows (its exp'd score columns are already in
matmul-lhsT layout), so no Vector work gates the kernel end; dummy warm-up
matmuls on memset data bridge the first-load DMA wait to keep the HAM clock
gate open.  HBM in is ~11MB/core; measured ~107us/core at the full 2.4GHz
PE clock (~85us of that is the bf16 matmul-streaming roofline).
"""

import sys
import numpy as np
import ml_dtypes

_BF16 = ml_dtypes.bfloat16

for _p in ("/opt/trn_rl_repo", "/root/.axon_site/_ro/trn_rl_repo"):
    if _p not in sys.path:
        sys.path.append(_p)

B, S, R, H = 32, 512, 196, 1024
NCORES = 8
BLOC = B // NCORES          # batches per core
HC = H // 128               # contraction chunks of 128
OC = 512                    # output-chunk (one fp32 PSUM bank)
NI = BLOC * R               # img rows, all batches packed (784)
ND = BLOC * S               # dns rows, all batches packed (2048)

_CACHE = {}


def _row_chunks(n):
    out, o = [], 0
    while o < n:
        out.append((o, min(128, n - o)))
        o += 128
    return out


def build_nc():
    from concourse import bacc, mybir
    from concourse import tile

    f32, f16 = mybir.dt.float32, mybir.dt.bfloat16
    Act = mybir.ActivationFunctionType
    Alu = mybir.AluOpType

    nc = bacc.Bacc("TRN2", target_bir_lowering=False, debug=False)

    xt_img = nc.dram_tensor("xt_img", [HC, 128, NI], f16, kind="ExternalInput")
    xt_dns = nc.dram_tensor("xt_dns", [HC, 128, ND], f16, kind="ExternalInput")
    wt_i1 = nc.dram_tensor("wt_i1", [HC, 128, H], f16, kind="ExternalInput")
    wt_d2 = nc.dram_tensor("wt_d2", [HC, 128, H], f16, kind="ExternalInput")
    wrow_b = nc.dram_tensor("wrow_b", [128, H], f16, kind="ExternalInput")
    wrow_d = nc.dram_tensor("wrow_d", [128, H], f16, kind="ExternalInput")
    ident_d = nc.dram_tensor("ident", [128, 128], f16, kind="ExternalInput")
    # [p, b, hc] layout: element h of batch b lives at [h % 128, b, h // 128];
    # contiguous 32B runs per partition (a [b, h] layout would be a 4-byte
    # scatter with 512B stride - ~9us of DMA RMW per batch on HW)
    xn_b3 = nc.dram_tensor("xn_b3", [S // 128, 128, H], f16, kind="ExternalInput")
    out_dns = nc.dram_tensor("out_dns", [128, BLOC, HC], f32, kind="ExternalOutput")
    out_img = nc.dram_tensor("out_img", [128, BLOC, HC], f32, kind="ExternalOutput")
    out_dns_b3 = nc.dram_tensor("out_dns_b3", [1, H], f32, kind="ExternalOutput")

    with tile.TileContext(nc) as tc:
        with (
            tc.tile_pool(name="const", bufs=1) as cpool,
            tc.tile_pool(name="work", bufs=3) as wpool,
            tc.tile_pool(name="small", bufs=8) as spool,
            tc.tile_pool(name="pp", bufs=2, space="PSUM") as ppool,
            tc.tile_pool(name="pt", bufs=1, space="PSUM") as ptps,
        ):
            xt_i = cpool.tile([128, HC * NI], f16, name="xt_img_sb")
            xt_d = cpool.tile([128, HC * ND], f16, name="xt_dns_sb")
            wt_sb = {"img": cpool.tile([128, HC * H], f16, name="wt_i1_sb"),
                     "dns": cpool.tile([128, HC * H], f16, name="wt_d2_sb")}
            wr_sb = {"img": cpool.tile([128, H], f16, name="wrow_b_sb"),
                     "dns": cpool.tile([128, H], f16, name="wrow_d_sb")}
            ident = cpool.tile([128, 128], f16, name="ident_sb")
            xn4 = cpool.tile([128, (S // 128) * H], f16, name="xn_b3_sb")
            ones_col = cpool.tile([128, 1], f16, name="ones_col")
            nc.vector.memset(ones_col[:, :], 1.0)
            # warm-up: dummy matmuls on memset data bridge the first-load DMA
            # wait, so the HAM clock-gate is already at 8/8 (2.4GHz) when the
            # first real projection group starts (see tensor-engine guide)
            warm = cpool.tile([128, 256], f16, name="warm_sb")
            nc.vector.memset(warm[:, :], 1.0)
            wps = ptps.tile([128, 512], f32, name="warm_ps", tag="abps",
                            bufs=4)
            for _ in range(12):
                nc.tensor.matmul(wps[:, 0:256], lhsT=warm[:, 0:128],
                                 rhs=warm[:, 0:256], start=True, stop=True)
            att_sb = {s: cpool.tile([128, BLOC * HC], f32, name=f"att_{s}_sb")
                      for s in ("img", "dns")}

            wt_dram = {"img": wt_i1, "dns": wt_d2}
            wr_dram = {"img": wrow_b, "dns": wrow_d}
            xt_dram = {"img": xt_img, "dns": xt_dns}
            out_d = {"img": out_img, "dns": out_dns}
            n_rows = {"img": R, "dns": S}

            def load_wt_hc(side, hc, eng=None):
                (eng or nc.sync).dma_start(
                    out=wt_sb[side][:, hc * H:(hc + 1) * H],
                    in_=wt_dram[side][hc])

            def load_xt(side, c0, c1, h0=0, h1=HC, eng=None):
                (eng or nc.sync).dma_start(
                    out=xt3[side][:, h0:h1, c0:c1],
                    in_=xt_dram[side][h0:h1, :, c0:c1]
                    .rearrange("hc p m -> p hc m"))

            wt3 = {s: wt_sb[s].rearrange("p (hc m) -> p hc m", hc=HC)
                   for s in ("img", "dns")}
            xt3 = {"img": xt_i.rearrange("p (hc m) -> p hc m", hc=HC),
                   "dns": xt_d.rearrange("p (hc m) -> p hc m", hc=HC)}

            # groups: (side, row0, row1, tail segments (batch, off, nr));
            # img packs all 4 batches (7 row chunks instead of 8)
            groups = [
                ("img", 0, NI, [(b, b * R, R) for b in range(BLOC)]),
                ("dns", 0 * S, 1 * S, [(0, 0, S)]),
                ("dns", 1 * S, 2 * S, [(1, 0, S)]),
                ("dns", 2 * S, 3 * S, [(2, 0, S)]),
                ("dns", 3 * S, 4 * S, [(3, 0, S)]),
            ]

            def emit_loads(gi):
                side = groups[gi][0]
                if gi == 0:
                    # ramp: interleave per-hc weight/activation chunks over
                    # two DGE queues so the first proj group starts early
                    for hc in range(HC):
                        load_wt_hc("img", hc,
                                   eng=nc.scalar if hc % 2 else nc.sync)
                        load_xt("img", 0, NI, hc, hc + 1,
                                eng=nc.sync if hc % 2 else nc.scalar)
                    nc.sync.dma_start(out=ident[:, :], in_=ident_d[:, :])
                    nc.sync.dma_start(out=wr_sb["img"][:, :],
                                      in_=wr_dram["img"][:, :])
                elif gi == 1:
                    load_xt("dns", 0, S)
                    for hc in range(HC):
                        load_wt_hc("dns", hc)
                    nc.sync.dma_start(out=wr_sb["dns"][:, :],
                                      in_=wr_dram["dns"][:, :])
                else:
                    g0, g1 = groups[gi][1], groups[gi][2]
                    load_xt("dns", g0, g1)
                    if gi == len(groups) - 2:
                        # natural-layout rows of the last batch for the
                        # PE-side stage-2, loaded a group early so the
                        # deferred per-chunk tail matmuls never wait
                        nc.sync.dma_start(
                            out=xn4.rearrange("p (rc m) -> p rc m", rc=S // 128),
                            in_=xn_b3.rearrange("rc p m -> p rc m"))

            def emit_group(gi, prev_tail):
                side, g0, g1, _ = groups[gi]
                rcs = _row_chunks(g1 - g0)
                acs = spool.tile([128, len(rcs)], f16, name=f"acs_{gi}",
                                 tag=f"acs_{side}", bufs=2)
                emit_loads(gi)
                last_grp = gi == len(groups) - 1
                tl = {}
                if last_grp:
                    # softmax-sum and stage-2 accumulators for the deferred
                    # per-chunk tail matmuls (one-bank tiles from the abps
                    # ring; the stage-2 accumulator is split in oc-halves)
                    tl["sps"] = ptps.tile([128, S], f32, name="sps_B",
                                          tag="abps", bufs=4)
                    tl["att"] = [ptps.tile([128, OC], f32, name=f"attps_B{o}",
                                           tag="abps", bufs=4)
                                 for o in range(2)]

                def tail_mms(ci, rk, nrc):
                    # accumulate the final batch's softmax sum and stage-2
                    # weighted row sum for chunk ci on the PE; emitted one
                    # chunk deferred so the score chain is already done
                    nc.tensor.matmul(
                        tl["sps"][0:1, 0:1], lhsT=acs[0:rk, ci:ci + 1],
                        rhs=ones_col[0:rk, 0:1],
                        start=(ci == 0), stop=(ci == nrc - 1))
                    for o2 in range(2):
                        nc.tensor.matmul(
                            tl["att"][o2][:, 0:OC],
                            lhsT=acs[0:rk, ci:ci + 1].to_broadcast((rk, 128)),
                            rhs=xn4.rearrange("p (rc m) -> p rc m",
                                              rc=S // 128)[0:rk, ci,
                                                           o2 * OC:(o2 + 1) * OC],
                            start=(ci == 0), stop=(ci == nrc - 1))

                last_chunk = False  # oc-half split of the final chunk
                # measured ~2us slower than the plain path; keep disabled
                for ci, (c0, rk) in enumerate(rcs):
                    r0 = g0 + c0
                    split = last_chunk and ci == len(rcs) - 1
                    ps = ppool.tile([128, H], f32, name=f"proj_{gi}_{ci}",
                                    tag="pp")
                    th = wpool.tile([128, H], f16, name=f"th_{gi}_{ci}",
                                    tag="th", bufs=3)
                    scr = wpool.tile([128, H], f16, name=f"scr_{gi}_{ci}",
                                     tag="scr", bufs=2)
                    tcol = spool.tile([128, 2], f32, name=f"tc_{gi}_{ci}",
                                      tag="tcol", bufs=4)

                    def half(o2):
                        sl = slice(o2 * OC, (o2 + 1) * OC)
                        nc.scalar.activation(th[0:rk, sl], ps[0:rk, sl],
                                             Act.Tanh)
                        nc.vector.scalar_tensor_tensor(
                            out=scr[0:rk, sl], in0=th[0:rk, sl], scalar=1.0,
                            in1=wr_sb[side][0:rk, sl],
                            op0=Alu.mult, op1=Alu.mult,
                            accum_out=tcol[0:rk, o2:o2 + 1])

                    if split:
                        # the kernel's final chunk: two oc-half accumulation
                        # groups so half the tanh/score chain overlaps the
                        # second half's matmuls, shortening the end chain
                        for o2 in range(2):
                            for hc in range(HC):
                                nc.tensor.matmul(
                                    ps[0:rk, o2 * OC:(o2 + 1) * OC],
                                    lhsT=xt3[side][:, hc, r0:r0 + rk],
                                    rhs=wt3[side][:, hc,
                                                  o2 * OC:(o2 + 1) * OC],
                                    start=(hc == 0), stop=(hc == HC - 1))
                            half(o2)
                        nc.vector.scalar_tensor_tensor(
                            out=tcol[0:rk, 0:1], in0=tcol[0:rk, 0:1],
                            scalar=1.0, in1=tcol[0:rk, 1:2],
                            op0=Alu.mult, op1=Alu.add)
                    else:
                        for hc in range(HC):
                            lhs = xt3[side][:, hc, r0:r0 + rk]
                            for o2 in range(2):
                                nc.tensor.matmul(
                                    ps[0:rk, o2 * OC:(o2 + 1) * OC],
                                    lhsT=lhs,
                                    rhs=wt3[side][:, hc,
                                                  o2 * OC:(o2 + 1) * OC],
                                    start=(hc == 0), stop=(hc == HC - 1))
                        nc.scalar.activation(th[0:rk, :], ps[0:rk, :],
                                             Act.Tanh)
                        nc.vector.scalar_tensor_tensor(
                            out=scr[0:rk, :], in0=th[0:rk, :], scalar=1.0,
                            in1=wr_sb[side][0:rk, :],
                            op0=Alu.mult, op1=Alu.mult,
                            accum_out=tcol[0:rk, 0:1])
                    nc.scalar.activation(acs[0:rk, ci:ci + 1],
                                         tcol[0:rk, 0:1], Act.Exp)
                    if last_grp and ci >= 1:
                        pc0, prk = rcs[ci - 1]
                        tail_mms(ci - 1, prk, len(rcs))
                    # spread the previous group's tail parts across this
                    # group's chunk slots so its vector work interleaves with
                    # (instead of delaying) this group's score reductions
                    if prev_tail is not None and ci >= 1:
                        take = (len(prev_tail) + len(rcs) - 2) // (len(rcs) - 1)
                        for part in prev_tail[(ci - 1) * take: ci * take]:
                            part()
                if prev_tail is not None:
                    nslots = len(rcs) - 1
                    take = (len(prev_tail) + nslots - 1) // nslots
                    for part in prev_tail[nslots * take:]:
                        part()
                if last_grp:
                    c0l, rkl = rcs[-1]
                    tail_mms(len(rcs) - 1, rkl, len(rcs))
                    return [lambda: emit_pe_tail(gi, tl)]
                return make_tail(gi, acs, rcs)

            def make_tail(gi, acs, rcs):
                side, g0, g1, segs = groups[gi]
                ng = g1 - g0
                parts = []
                a_b = wpool.tile([128, ng], f16, name=f"ab_{gi}",
                                 tag=f"ab_{side}", bufs=1 if side == "img" else 2)

                def bcast():
                    # broadcast each exp'd score column to [128, rk] rows via
                    # a tiny PE matmul (a-col x identity); one PSUM bank per
                    # pass so the proj pool can triple-buffer
                    for p in range((ng + 511) // 512):
                        p0, p1 = p * 512, min(ng, p * 512 + 512)
                        ab_ps = ptps.tile([128, 512], f32,
                                          name=f"abps_{gi}_{p}", tag="abps",
                                          bufs=4)
                        for ci, (c0, rk) in enumerate(rcs):
                            if p0 <= c0 < p1:
                                nc.tensor.matmul(
                                    ab_ps[:, c0 - p0:c0 - p0 + rk],
                                    lhsT=acs[0:rk, ci:ci + 1]
                                    .to_broadcast((rk, 128)),
                                    rhs=ident[0:rk, 0:rk],
                                    start=True, stop=True)
                        nc.vector.tensor_copy(a_b[:, p0:p1],
                                              ab_ps[:, 0:p1 - p0])
                parts.append(bcast)
                for b, off, nr in segs:
                    parts.append(lambda b=b, off=off, nr=nr:
                                 tail_seg(gi, a_b, b, off, nr))
                return parts

            def tail_seg(gi, a_b, b, off, nr):
                side, g0, g1, segs = groups[gi]
                if True:
                    # softmax sum: every partition of a_b holds the full
                    # weight row, so a Copy+accum gives the sum broadcast
                    scrap = wpool.tile([128, n_rows[side]], f16,
                                       name=f"scrap_{gi}_{b}", tag="scrap",
                                       bufs=2)
                    asum = spool.tile([128, 1], f32, name=f"as_{gi}_{b}",
                                      tag="asum", bufs=4)
                    nc.scalar.activation(scrap[:, 0:nr], a_b[:, off:off + nr],
                                         Act.Copy, accum_out=asum[:, 0:1])
                    rb = spool.tile([128, 1], f32, name=f"rb_{gi}_{b}",
                                    tag="rb", bufs=4)
                    nc.vector.reciprocal(rb[:, 0:1], asum[:, 0:1])
                    attc = spool.tile([128, HC], f32, name=f"attc_{gi}_{b}",
                                      tag="attc", bufs=2)
                    scr2 = wpool.tile([128, n_rows[side]], f16,
                                      name=f"sc2_{gi}_{b}", tag="scr2", bufs=2)
                    for hc in range(HC):
                        nc.vector.scalar_tensor_tensor(
                            out=scr2[:, 0:nr],
                            in0=xt3[side][:, hc, g0 + off:g0 + off + nr],
                            scalar=1.0, in1=a_b[:, off:off + nr],
                            op0=Alu.mult, op1=Alu.mult,
                            accum_out=attc[:, hc:hc + 1])
                    nc.scalar.activation(
                        att_sb[side][:, b * HC:(b + 1) * HC], attc[:, 0:HC],
                        Act.Copy, scale=rb[:, 0:1])
                    if side == "img" and b == BLOC - 1:
                        nc.sync.dma_start(
                            out=out_d[side].rearrange("p b hc -> p (b hc)"),
                            in_=att_sb[side][:, :])
                    if side == "dns" and b == BLOC - 2:
                        nc.sync.dma_start(
                            out=out_d[side][:, 0:BLOC - 1, :]
                            .rearrange("p b hc -> p (b hc)"),
                            in_=att_sb[side][:, 0:(BLOC - 1) * HC])

            def emit_pe_tail(gi, tl):
                """Final-batch epilogue: the sums/stage-2 matmuls already ran
                inside the projection stream; only normalize and write."""
                r1 = spool.tile([1, 1], f32, name="r1_B", tag="r1_B")
                nc.vector.reciprocal(r1[0:1, 0:1], tl["sps"][0:1, 0:1])
                att_row = wpool.tile([1, H], f32, name="attrow_B", tag="attrow")
                for o2 in range(2):
                    nc.scalar.activation(att_row[0:1, o2 * OC:(o2 + 1) * OC],
                                         tl["att"][o2][0:1, 0:OC],
                                         Act.Copy, scale=r1[0:1, 0:1])
                nc.sync.dma_start(out=out_dns_b3[:, :], in_=att_row[0:1, :])

            tail = None
            for gi in range(len(groups)):
                tail = emit_group(gi, tail)
            for part in tail:
                part()
    nc.compile()
    return nc


def _get_nc():
    if "nc" not in _CACHE:
        _CACHE["nc"] = build_nc()
    return _CACHE["nc"]


def make_in_maps(inputs):
    dns = np.ascontiguousarray(np.asarray(inputs["dns_feature"], dtype=np.float32))
    img = np.ascontiguousarray(np.asarray(inputs["img_features"], dtype=np.float32))
    W_i1 = np.asarray(inputs["W_i1"], dtype=np.float32)
    W_d2 = np.asarray(inputs["W_d2"], dtype=np.float32)
    wB = np.asarray(inputs["w_att1"], dtype=np.float32)[H:]
    wD = np.asarray(inputs["w_att2"], dtype=np.float32)[H:]

    wt_i1 = np.ascontiguousarray(W_i1.T).reshape(HC, 128, H).astype(_BF16)
    wt_d2 = np.ascontiguousarray(W_d2.T).reshape(HC, 128, H).astype(_BF16)
    wrow_b = np.ascontiguousarray(np.broadcast_to(wB, (128, H))).astype(_BF16)
    wrow_d = np.ascontiguousarray(np.broadcast_to(wD, (128, H))).astype(_BF16)
    ident = np.eye(128, dtype=_BF16)

    in_maps = []
    for k in range(NCORES):
        sl = slice(k * BLOC, (k + 1) * BLOC)
        # [BLOC, rows, H] -> [H, BLOC*rows] -> [HC, 128, n]
        xt_d = dns[sl].reshape(BLOC * S, H).T.reshape(HC, 128, BLOC * S)
        xt_i = img[sl].reshape(BLOC * R, H).T.reshape(HC, 128, BLOC * R)
        in_maps.append({
            "xn_b3": np.ascontiguousarray(
                dns[sl][BLOC - 1].reshape(S // 128, 128, H)).astype(_BF16),
            "xt_dns": np.ascontiguousarray(xt_d).astype(_BF16),
            "xt_img": np.ascontiguousarray(xt_i).astype(_BF16),
            "wt_i1": wt_i1,
            "wt_d2": wt_d2,
            "wrow_b": wrow_b,
            "wrow_d": wrow_d,
            "ident": ident,
        })
    return in_maps


def kernel(**inputs):
    from concourse.bass_utils import run_bass_kernel_spmd

    nc = _get_nc()
    in_maps = make_in_maps(inputs)
    res = run_bass_kernel_spmd(nc, in_maps, list(range(NCORES))).results
    # device out: [128, BLOC, HC], element h of batch b at [h % 128, b, h//128]
    outs = {}
    for name in ("out_dns", "out_img"):
        per = []
        for k in range(NCORES):
            a = res[k][name].transpose(1, 2, 0).reshape(BLOC, H).copy()
            if name == "out_dns":
                a[BLOC - 1] = res[k]["out_dns_b3"][0]
            per.append(a)
        outs[name] = np.concatenate(per, axis=0)
    out_dns = np.ascontiguousarray(
        np.broadcast_to(outs["out_dns"][:, None, :], (B, S, H)))
    out_img = np.ascontiguousarray(
        np.broadcast_to(outs["out_img"][:, None, :], (B, S, H)))
    return out_dns, out_img
